# revision 31
# baseline (speedup 1.0000x reference)
"""Trainium2 Bass kernel for nn_ContextDrivingForce (dense MLP, 3 fused layers).

Math (per token row, D=896):
    u_proj = u @ W_a.T + b_a
    alpha  = sigmoid(sum(h * u_proj) / sqrt(D))
    u_att  = alpha * u
    g      = sigmoid([h, u_att] @ W_g.T + b_g)
    u_gate = g * u_att
    out    = gelu([h, u_gate, h*u_gate] @ W_f.T + b_f)        (exact erf gelu)

Distribution: data-parallel over tokens across 8 NeuronCores, weights
replicated. Device tensors feature-major ([D, tokens]); host transposes.

Precision strategy (validated by a host-side elementwise simulation of the
exact device dataflow; sim matched HW to 5 digits on previous revisions):
  - Layers 1+2 matmuls fp8e4m3 with DoubleRow (2 k-tiles per MM issue).
    Their output error is damped by the sigmoid gates (|sigma'| <= 1/4).
  - Layer 3: the h-part (87% of z3 variance) stays bf16; the ug/hu parts
    run fp8+DoubleRow.
  - The DVE product chain uses clean bf16 copies (u_bf -> ua_bf -> ug_bf)
    so fp8 rounding does not compound through the products.
  - Activations carry embedded scale S_ACT=4, weights S_W=64 (keeps fp8
    operands in the normal range; all folds are powers of two).

Schedule: chunks of T=512 tokens, software-pipelined so the PE never waits
on the logit -> tanh -> broadcast -> DVE chain: chunk c+1's layer-1 matmuls
are emitted between chunk c's broadcast and layer 2. All input DMA rides
the sync HWDGE ring in hand-tuned arrival order (the GPSIMD ring is a slow
software-dynamic queue -- do not use it); output stores ride the ACT ring.

Sigmoids via tanh (sigmoid(x) = (tanh(x/2)+1)/2) so every ACT op uses the
single `gelu_and_others` table set; factors of 2 are folded into host-side
weight scaling (W_g ua-cols x0.5, W_f ug/hu-cols x0.25).
"""

import math
import sys
from contextlib import ExitStack

for _p in ("/root/.axon_site", "/root/.axon_site/_ro/trn_rl_repo"):
    if _p not in sys.path:
        sys.path.append(_p)

import ml_dtypes
import numpy as np

import concourse.bass as bass
import concourse.mybir as mybir
import concourse.tile as tile
from concourse import bacc
from concourse.bass_utils import run_bass_kernel_spmd

P = 128
D = 896
KD = D // P  # 7 feature tiles
N_TOK = 16384
N_CORES = 8
NPC = N_TOK // N_CORES  # 2048 tokens per core

F32 = mybir.dt.float32
BF16 = mybir.dt.bfloat16
F8 = mybir.dt.float8e4
AF = mybir.ActivationFunctionType
ALU = mybir.AluOpType
DR = mybir.MatmulPerfMode.DoubleRow

S_ACT = 4.0     # embedded scale on h/u and all derived activations
S_W = 64.0      # embedded scale on all weights
S1 = S_ACT * S_W


def build_nc(npc=NPC, T=512, l3fp8=True, mm_bufs=7, act_bufs=2):
    n_chunks = npc // T
    assert n_chunks * T == npc

    nc = bacc.Bacc()
    hbf_d = nc.declare_dram_parameter("hbf", [P, n_chunks, KD, T], BF16,
                                      isOutput=False)
    ubf_d = nc.declare_dram_parameter("ubf", [P, n_chunks, KD, T], BF16,
                                      isOutput=False)
    h8_d = nc.declare_dram_parameter("h8", [P, n_chunks, KD, T], F8, isOutput=False)
    u8_d = nc.declare_dram_parameter("u8", [P, n_chunks, KD, T], F8, isOutput=False)
    wa_d = nc.declare_dram_parameter("wa", [P, KD, D], F8, isOutput=False)
    # wg k-order: [ua-cols (folded 0.5) | h-cols] to match the uah tile
    wg_d = nc.declare_dram_parameter("wg", [P, 2 * KD, D], F8, isOutput=False)
    wfh_d = nc.declare_dram_parameter("wfh", [P, KD, D], BF16, isOutput=False)
    l3dt = F8 if l3fp8 else BF16
    wfuh_d = nc.declare_dram_parameter("wfuh", [P, 2 * KD, D], l3dt, isOutput=False)
    bias_d = nc.declare_dram_parameter("biasp", [P, 3 * KD], F32, isOutput=False)
    gT_d = nc.declare_dram_parameter("gT", [D, npc], BF16, isOutput=True)

    inv_sqrt_d = 1.0 / math.sqrt(D)

    with tile.TileContext(nc) as tc, ExitStack() as ctx:
        wp = ctx.enter_context(tc.tile_pool(name="weights", bufs=1))
        hbp = ctx.enter_context(tc.tile_pool(name="hbp", bufs=act_bufs))
        ubp = ctx.enter_context(tc.tile_pool(name="ubp", bufs=act_bufs))
        up = ctx.enter_context(tc.tile_pool(name="up", bufs=3))
        uahp = ctx.enter_context(tc.tile_pool(name="uahp", bufs=3))
        uabp = ctx.enter_context(tc.tile_pool(name="uabp", bufs=act_bufs))
        ughup = ctx.enter_context(tc.tile_pool(name="ughup", bufs=act_bufs))
        ugbp = ctx.enter_context(tc.tile_pool(name="ugbp", bufs=act_bufs))
        tmpp = ctx.enter_context(tc.tile_pool(name="tmpp", bufs=act_bufs))
        sp = ctx.enter_context(tc.tile_pool(name="small", bufs=3))
        op = ctx.enter_context(tc.tile_pool(name="outp", bufs=3))
        pp = ctx.enter_context(tc.tile_pool(name="psum", bufs=1, space="PSUM"))

        bias_sb = wp.tile([P, 3 * KD], F32, name="biasp")
        ones_col = wp.tile([P, 1], BF16, name="ones_col")
        nc.vector.memset(ones_col, 1.0)
        ones_row = wp.tile([1, P], BF16, name="ones_row")
        nc.vector.memset(ones_row, 1.0)

        # ---- SBUF weight tiles
        wa_sb = wp.tile([P, KD, D], F8, name="wa")
        wg_sb = wp.tile([P, 2 * KD, D], F8, name="wg")
        wfh_sb = wp.tile([P, KD, D], BF16, name="wfh")
        wfuh_sb = wp.tile([P, 2 * KD, D], l3dt, name="wfuh")

        # per-chunk input tiles, allocated lazily
        tiles = {}

        def alloc_chunk(c):
            tiles[c] = {
                "u8": up.tile([P, KD, T], F8, name=f"u8_{c}", tag="u8"),
                "uah": uahp.tile([P, 2 * KD, T], F8, name=f"uah{c}", tag="uah"),
                "hbf": hbp.tile([P, KD, T], BF16, name=f"hbf{c}", tag="hbf"),
                "ubf": ubp.tile([P, KD, T], BF16, name=f"ubf{c}", tag="ubf"),
                "tmp": tmpp.tile([P, KD, T], BF16, name=f"tmp{c}", tag="tmp"),
            }
            return tiles[c]

        # ---- prelude DMA, hand-ordered for earliest compute start.
        # sync ring is FIFO: arrival order == trigger order.
        alloc_chunk(0)
        alloc_chunk(1)
        nc.sync.dma_start(bias_sb, bias_d[:, :])
        # first k-pair split by m so the very first matmuls gate on ~100KB
        nc.sync.dma_start(wa_sb[:, 0:2, :256], wa_d[:, 0:2, :256])
        nc.sync.dma_start(tiles[0]["u8"][:, 0:2], u8_d[:, 0, 0:2])
        nc.sync.dma_start(wa_sb[:, 0:2, 256:], wa_d[:, 0:2, 256:])
        nc.sync.dma_start(wa_sb[:, 2:], wa_d[:, 2:])
        nc.sync.dma_start(tiles[0]["u8"][:, 2:], u8_d[:, 0, 2:])
        nc.sync.dma_start(tiles[0]["uah"][:, KD:], h8_d[:, 0])
        nc.sync.dma_start(tiles[1]["u8"], u8_d[:, 1])
        nc.sync.dma_start(tiles[1]["uah"][:, KD:], h8_d[:, 1])
        nc.sync.dma_start(wg_sb, wg_d[:, :])
        nc.sync.dma_start(tiles[0]["ubf"], ubf_d[:, 0])
        nc.sync.dma_start(tiles[0]["hbf"], hbf_d[:, 0])
        # split so chunk-0's L3 h-sweep / DR-sweep can start on the earlier
        # k-tiles while the rest is still in flight
        nc.sync.dma_start(wfh_sb[:, :4], wfh_d[:, :4])
        nc.sync.dma_start(wfh_sb[:, 4:], wfh_d[:, 4:])
        nc.sync.dma_start(wfuh_sb[:, :8], wfuh_d[:, :8])
        nc.sync.dma_start(wfuh_sb[:, 8:], wfuh_d[:, 8:])

        def emit_l1(c):
            """L1: per m-tile 3 DR pairs + odd k into one psum, then the
            fused (psum + S1*b_a) * h8 DVE op, then the DVE reduce tree.
            tmp uses the fp8 h copy so the bf16 h is off the critical path.
            Chunk 0 runs k-pair-outer (grouped over 4/3 m-tiles) so compute
            starts as soon as the first k-pair lands from the DMA prelude;
            later chunks run m-major (lowest PSUM footprint)."""
            t = tiles[c]
            u8, uah, tmp = t["u8"], t["uah"], t["tmp"]
            if c == 0:
                for grp in (range(0, 4), range(4, KD)):
                    pss = {m: pp.tile([P, T], F32, name=f"ps1_{c}_{m}",
                                      tag="mm", bufs=mm_bufs) for m in grp}
                    for k in range(0, KD - 1, 2):
                        for m in grp:
                            nc.tensor.matmul(
                                pss[m], lhsT=wa_sb[:, k:k + 2, m * P:(m + 1) * P],
                                rhs=u8[:, k:k + 2, :], start=(k == 0),
                                stop=False, perf_mode=DR)
                    for m in grp:
                        nc.tensor.matmul(pss[m],
                                         lhsT=wa_sb[:, KD - 1, m * P:(m + 1) * P],
                                         rhs=u8[:, KD - 1, :], start=False,
                                         stop=True)
                    for m in grp:
                        nc.vector.scalar_tensor_tensor(
                            out=tmp[:, m, :], in0=pss[m],
                            scalar=bias_sb[:, m:m + 1],
                            in1=uah[:, KD + m, :], op0=ALU.add, op1=ALU.mult)
            else:
                for m in range(KD):
                    ps = pp.tile([P, T], F32, name=f"ps1_{c}_{m}", tag="mm",
                                 bufs=mm_bufs)
                    for k in range(0, KD - 1, 2):
                        nc.tensor.matmul(ps, lhsT=wa_sb[:, k:k + 2, m * P:(m + 1) * P],
                                         rhs=u8[:, k:k + 2, :], start=(k == 0),
                                         stop=False, perf_mode=DR)
                    nc.tensor.matmul(ps, lhsT=wa_sb[:, KD - 1, m * P:(m + 1) * P],
                                     rhs=u8[:, KD - 1, :], start=False, stop=True)
                    nc.vector.scalar_tensor_tensor(
                        out=tmp[:, m, :], in0=ps, scalar=bias_sb[:, m:m + 1],
                        in1=uah[:, KD + m, :], op0=ALU.add, op1=ALU.mult)
            # partition-reduce prep: 7 -> 1 tile (saves 6 PE matmuls); runs on
            # the otherwise-idle GPSIMD engine, one section ahead of its
            # reduce matmul, so its latency never matters.
            s0 = sp.tile([P, T], BF16, name=f"s0_{c}", tag="tree", bufs=10)
            s1 = sp.tile([P, T], BF16, name=f"s1_{c}", tag="tree", bufs=10)
            s2 = sp.tile([P, T], BF16, name=f"s2_{c}", tag="tree", bufs=10)
            s3 = sp.tile([P, T], BF16, name=f"s3_{c}", tag="tree", bufs=10)
            s4 = sp.tile([P, T], BF16, name=f"s4_{c}", tag="tree", bufs=10)
            nc.gpsimd.tensor_add(out=s0, in0=tmp[:, 0, :], in1=tmp[:, 1, :])
            nc.gpsimd.tensor_add(out=s1, in0=tmp[:, 2, :], in1=tmp[:, 3, :])
            nc.gpsimd.tensor_add(out=s2, in0=tmp[:, 4, :], in1=tmp[:, 5, :])
            nc.gpsimd.tensor_add(out=s3, in0=s0, in1=s1)
            nc.gpsimd.tensor_add(out=s4, in0=s2, in1=tmp[:, 6, :])
            rsum = sp.tile([P, T], BF16, name=f"rs_{c}", tag="rsum", bufs=2)
            nc.gpsimd.tensor_add(out=rsum, in0=s3, in1=s4)
            t["rsum"] = rsum

        # ---- PE warmup: full-K dummy matmuls on memset operands run during
        # the DMA bring-up window so HAM unthrottles before real work arrives.
        # (Rank-1 dummies do NOT register as PE activity -- K must be 128.)
        wdum = sp.tile([P, P], BF16, name="wdum", tag="wdum", bufs=1)
        nc.vector.memset(wdum, 0.0)
        wsrc = sp.tile([P, T], BF16, name="wsrc", tag="wsrc", bufs=1)
        nc.vector.memset(wsrc, 0.0)
        wone = sp.tile([1, T], BF16, name="wone", tag="wone", bufs=1)
        nc.vector.memset(wone, 1.0)
        warm = pp.tile([P, T], F32, name="warm", tag="rab", bufs=1)
        for _ in range(10):
            nc.tensor.matmul(warm, lhsT=wdum, rhs=wsrc, start=True, stop=True)

        emit_l1(0)

        def emit_red(c):
            """Logit reduce MM + tanh. red shares a PSUM bank with the alpha
            broadcast (partition 0), freeing a bank for mm_bufs=7."""
            rab = pp.tile([P, T], F32, name=f"rab{c}", tag="rab", bufs=1)
            nc.tensor.matmul(rab[0:1, :], lhsT=ones_col, rhs=tiles[c]["rsum"],
                             start=True, stop=True)
            alpha = sp.tile([1, T], BF16, name=f"al{c}", tag="alpha", bufs=2)
            nc.scalar.activation(alpha, rab[0:1, :], AF.Tanh,
                                 scale=inv_sqrt_d * 0.5 / (S1 * S_ACT))
            tiles[c]["rab"] = rab
            tiles[c]["alpha"] = alpha

        def emit_ab(c):
            # broadcast (alpha'+1) to all partitions: alpha' plus an
            # accumulated ones row (overwrites the logit row, already
            # consumed by tanh). Holding the +1 in the psum turns the ua
            # STTs into plain tensor-muls, so the bf16 copy can run on the
            # otherwise-idle GPSIMD engine.
            nc.tensor.matmul(tiles[c]["rab"], lhsT=ones_row,
                             rhs=tiles[c]["alpha"], start=True, stop=False)
            nc.tensor.matmul(tiles[c]["rab"], lhsT=ones_row,
                             rhs=wone, start=False, stop=True)
            ab1 = sp.tile([P, T], BF16, name=f"ab1_{c}", tag="ab1", bufs=2)
            nc.vector.tensor_copy(out=ab1, in_=tiles[c]["rab"])
            tiles[c]["ab1"] = ab1

        emit_red(0)
        emit_ab(0)

        for c in range(n_chunks):
            cs = bass.ds(c * T, T)
            t = tiles.pop(c)
            u8, uah, hbf, ubf, ab = t["u8"], t["uah"], t["hbf"], t["ubf"], t["rab"]

            # ua' = (alpha'+1)*u = rab * u (the +1 lives in the psum).
            # fp8 copy (L2 rhs) on DVE, k=6 first: L2 consumes the cross
            # pair (ua6,h0) right after the DVE-free h pairs.
            for k in [KD - 1] + list(range(KD - 1)):
                nc.vector.tensor_mul(out=uah[:, k, :], in0=ab, in1=u8[:, k, :])
            # clean bf16 ua copy on GPSIMD (frees 7 DVE ops/section); feeds
            # the ug/hu DVE products, trickling in ahead of their needs
            ua_bf = uabp.tile([P, KD, T], BF16, name=f"uab{c}", tag="uab")
            for k in range(KD):
                nc.gpsimd.tensor_mul(out=ua_bf[:, k, :], in0=t["ab1"],
                                     in1=ubf[:, k, :])

            # next chunk's L1 fills the PE while ACT/DVE produce alpha & ua;
            # chunk c+2's fp8 inputs + c+1's bf16 inputs stream behind it
            if c + 1 < n_chunks:
                if c + 2 < n_chunks:
                    nt = alloc_chunk(c + 2)
                    nc.sync.dma_start(nt["u8"], u8_d[:, c + 2])
                    nc.sync.dma_start(nt["uah"][:, KD:], h8_d[:, c + 2])
                nc.sync.dma_start(tiles[c + 1]["ubf"], ubf_d[:, c + 1])
                nc.sync.dma_start(tiles[c + 1]["hbf"], hbf_d[:, c + 1])
                emit_l1(c + 1)

            # ---- layer 2: z2 = [ua', h] @ wg (7 DR pairs over uah).
            # m-major so each m's t2 -> ug -> hu chain starts as early as
            # possible: the ughu tile then completes ~4us/section sooner,
            # which is exactly when L3's DR pairs need it.
            ughu = ughup.tile([P, 2 * KD, T], l3dt, name=f"ughu{c}", tag="ughu")
            # DR pair order: h-side pairs first (DMA-fed, no DVE dependency),
            # then the cross pair (ua6,h0), then the ua pairs -- so the m=0
            # matmuls never wait on the ua STT stream.
            L2_PAIRS = [8, 10, 12, 6, 0, 2, 4]
            for m in range(KD):
                ps = pp.tile([P, T], F32, name=f"ps2_{c}_{m}", tag="mm",
                             bufs=mm_bufs)
                for j, k in enumerate(L2_PAIRS):
                    nc.tensor.matmul(ps, lhsT=wg_sb[:, k:k + 2, m * P:(m + 1) * P],
                                     rhs=uah[:, k:k + 2, :], start=(j == 0),
                                     stop=(j == len(L2_PAIRS) - 1), perf_mode=DR)
                t2 = sp.tile([P, T], BF16, name=f"t2_{c}_{m}", tag="t2",
                             bufs=KD)
                nc.scalar.activation(t2, ps, AF.Tanh,
                                     bias=bias_sb[:, KD + m:KD + m + 1],
                                     scale=0.5 / S1)
                # ug' = (t2+1)*ua'; hu' = h*ug' (from the fp8 ug: costs
                # ~1.4e-3 rel err vs a clean bf16 copy, saves 7 DVE ops)
                nc.vector.scalar_tensor_tensor(
                    out=ughu[:, m, :], in0=t2, scalar=1.0,
                    in1=ua_bf[:, m, :], op0=ALU.add, op1=ALU.mult)
                nc.vector.scalar_tensor_tensor(
                    out=ughu[:, KD + m, :], in0=hbf[:, m, :],
                    scalar=1.0 / S_ACT, in1=ughu[:, m, :],
                    op0=ALU.mult, op1=ALU.mult)

            # next chunk's logit reduce + tanh run behind L3(c) on PE/ACT so
            # the red->tanh->broadcast chain is fully hidden
            if c + 1 < n_chunks:
                emit_red(c + 1)

            # ---- layer 3: out = gelu([h]bf16 + [ug', hu']fp8-DR + b_f).
            # All 49 h-part matmuls first (no DVE dependency -- they fill the
            # window while the DVE finishes the ug/hu chain), then the DR
            # sweep. Holds all 7 psum banks of the mm tag simultaneously.
            pss3 = {}
            for m in range(KD):
                ps = pss3[m] = pp.tile([P, T], F32, name=f"ps3_{c}_{m}",
                                       tag="mm", bufs=mm_bufs)
                for k in range(KD):
                    nc.tensor.matmul(ps, lhsT=wfh_sb[:, k, m * P:(m + 1) * P],
                                     rhs=hbf[:, k, :], start=(k == 0), stop=False)
            for m in range(KD):
                ps = pss3[m]
                if l3fp8:
                    for k in range(0, 2 * KD, 2):
                        nc.tensor.matmul(ps, lhsT=wfuh_sb[:, k:k + 2, m * P:(m + 1) * P],
                                         rhs=ughu[:, k:k + 2, :], start=False,
                                         stop=(k == 2 * KD - 2), perf_mode=DR)
                else:
                    for k in range(2 * KD):
                        nc.tensor.matmul(ps, lhsT=wfuh_sb[:, k, m * P:(m + 1) * P],
                                         rhs=ughu[:, k, :], start=False,
                                         stop=(k == 2 * KD - 1))
                outp = op.tile([P, T], BF16, name=f"o{c}_{m}", tag="out")
                nc.scalar.activation(outp, ps, AF.Gelu,
                                     bias=bias_sb[:, 2 * KD + m:2 * KD + m + 1],
                                     scale=1.0 / S1)
                # store triggers ride the (idle by now) sync ring so their
                # ~0.6us trigger cost doesn't serialize behind gelus on ACT
                nc.sync.dma_start(gT_d[m * P:(m + 1) * P, cs], outp)

            if c + 1 < n_chunks:
                emit_ab(c + 1)
    nc.compile()
    return nc


def prep_inputs(h_t, u_t, W_a_w, W_a_b, W_g_w, W_g_b, W_f_w, W_f_b,
                npc=NPC, T=512, l3fp8=True):
    """Host-side: transpose to feature-major, fold scales, quantize, shard."""
    f8 = ml_dtypes.float8_e4m3
    bf16 = ml_dtypes.bfloat16

    h = np.asarray(h_t, np.float32)
    u = np.asarray(u_t, np.float32)
    Wa = np.asarray(W_a_w, np.float32)
    Wg = np.asarray(W_g_w, np.float32)
    Wf = np.asarray(W_f_w, np.float32)
    ba = np.asarray(W_a_b, np.float32)
    bg = np.asarray(W_g_b, np.float32)
    bf = np.asarray(W_f_b, np.float32)

    waT = S_W * Wa.T
    wgT = S_W * np.concatenate([Wg[:, D:] * 0.5, Wg[:, :D]], axis=1).T
    wfhT = S_W * Wf[:, :D].T
    wfuhT = S_W * np.concatenate([Wf[:, D:2 * D] * 0.25, Wf[:, 2 * D:] * 0.25],
                                 axis=1).T

    def wpack(w, dt):  # [K_in, D_out] -> [128, K_in/128, D_out]
        return np.ascontiguousarray(
            w.reshape(-1, P, D).transpose(1, 0, 2)).astype(dt)

    wa_p = wpack(waT, f8)
    wg_p = wpack(wgT, f8)
    wfh_p = wpack(wfhT, bf16)
    wfuh_p = wpack(wfuhT, f8 if l3fp8 else bf16)
    biasp = np.ascontiguousarray(
        np.concatenate([S1 * ba, 0.5 * bg, bf]).reshape(3 * KD, P).T
    ).astype(np.float32)

    nch = npc // T

    def xpack(x, i, dt):  # x [N, D] -> [P, nch, KD, T] for core i
        blk = x[i * npc:(i + 1) * npc]                    # [npc, D]
        blk = blk.reshape(nch, T, KD, P)                  # [c, t, k, p]
        return np.ascontiguousarray(blk.transpose(3, 0, 2, 1)).astype(dt)

    hs = S_ACT * h
    us = S_ACT * u
    n_cores = h.shape[0] // npc
    in_maps = []
    for i in range(n_cores):
        in_maps.append({
            "hbf": xpack(hs, i, bf16), "ubf": xpack(us, i, bf16),
            "h8": xpack(hs, i, f8), "u8": xpack(us, i, f8),
            "wa": wa_p, "wg": wg_p, "wfh": wfh_p, "wfuh": wfuh_p,
            "biasp": biasp,
        })
    return in_maps


_NC_CACHE = {}


def _get_nc(npc=NPC, T=512, l3fp8=True):
    key = (npc, T, l3fp8)
    if key not in _NC_CACHE:
        _NC_CACHE[key] = build_nc(npc=npc, T=T, l3fp8=l3fp8)
    return _NC_CACHE[key]


def run(inputs, npc=NPC, T=512, l3fp8=True, clean_hu=True, trace=False, **kw):
    """Run the SPMD kernel; returns (full fp32 [N,D] output, BassKernelResults)."""
    nc = _get_nc(npc=npc, T=T, l3fp8=l3fp8)
    in_maps = prep_inputs(
        inputs["h_t"], inputs["u_t"], inputs["W_a_w"], inputs["W_a_b"],
        inputs["W_g_w"], inputs["W_g_b"], inputs["W_f_w"], inputs["W_f_b"],
        npc=npc, T=T, l3fp8=l3fp8)
    res = run_bass_kernel_spmd(nc, in_maps, list(range(len(in_maps))),
                               trace=trace, **kw)
    out = np.concatenate(
        [np.asarray(r["gT"]).astype(np.float32).T for r in res.results], axis=0)
    return out, res


def kernel(h_t, u_t, token_idx, u_all, W_a_w, W_a_b, W_g_w, W_g_b, W_f_w, W_f_b):
    # token_idx / u_all are unused by the reference math.
    inputs = {"h_t": h_t, "u_t": u_t, "W_a_w": W_a_w, "W_a_b": W_a_b,
              "W_g_w": W_g_w, "W_g_b": W_g_b, "W_f_w": W_f_w, "W_f_b": W_f_b}
    out, _ = run(inputs)
    return out
